# revision 3
# baseline (speedup 1.0000x reference)
"""Causal attention (B=4, S=4096, D_IN=768, D_OUT=64) on 8 Trainium2 NeuronCores.

Sharding: core c handles batch b=c//2 and key-parity p=c%2 (the even or odd
128-wide key tiles of that batch). Every core computes, for ALL queries of its
batch, the unnormalized attention partials over its own key set:
    num[o, q] = sum_{k in own} exp(q.k/8) * V[k, o]
    den[q]    = sum_{k in own} exp(q.k/8)
The host sums the two partials per batch and normalizes: ctx = (num/den).T.
Causality is exact: key-tile work is skipped below the diagonal band and the
two boundary blocks are masked with host-provided mask tiles.

Host prep per core: x[b].T with columns permuted to [own key tiles | other key
tiles] so the device program is identical across cores (SPMD); masks and a
64x64 identity (for the on-chip V^T -> V transpose) are passed as inputs.

All matmuls run in float32r (single-pass fp32 on the PE at bf16 rate for
moving dims >= 256; ~1e-4 relative accuracy).
"""
import numpy as np

import concourse.bass as bass
import concourse.bacc as bacc
import concourse.tile as tile
from concourse import mybir
from concourse.bass_utils import run_bass_kernel_spmd

B, S, DI, DO = 4, 4096, 768, 64
NCORES = 8
NIC = DI // 128          # 6 contraction chunks
NKT = S // 128           # 32 global key tiles per batch
NOWN = NKT // 2          # 16 own key tiles per core
QT = 512                 # query tile width (one PSUM bank of fp32)
NQT = S // QT            # 8 query tiles
F32 = mybir.dt.float32
F32R = mybir.dt.float32r

_prog_cache = {}


def j0_of(T):
    """First diagonal-region packed key tile for permuted query tile T."""
    return 4 * T if T < 4 else 4 * (T - 4)


def build_program():
    """Build + compile the single SPMD Bass program (identical on all cores)."""
    nc = bacc.Bacc("TRN2", target_bir_lowering=False, debug=False)

    xT = nc.declare_dram_parameter("xT", [DI, S], F32R, isOutput=False)
    wkv = nc.declare_dram_parameter("wkv", [DI, 128], F32R, isOutput=False)
    wq = nc.declare_dram_parameter("wq", [DI, DO], F32R, isOutput=False)
    mdiag = nc.declare_dram_parameter("mdiag", [128, 128], F32R, isOutput=False)
    mpcol = nc.declare_dram_parameter("mpcol", [128, 128], F32R, isOutput=False)
    ident = nc.declare_dram_parameter("ident", [DO, DO], F32R, isOutput=False)
    nd = nc.declare_dram_parameter("nd", [DO + 1, S], F32, isOutput=True)

    with tile.TileContext(nc) as tc:
        with tc.tile_pool(name="consts", bufs=1) as consts, \
             tc.tile_pool(name="xpool", bufs=1) as xpool, \
             tc.tile_pool(name="qkv", bufs=1) as qkv, \
             tc.tile_pool(name="expp", bufs=4) as expp, \
             tc.tile_pool(name="ndst", bufs=3) as ndst, \
             tc.tile_pool(name="ps_proj", bufs=2, space="PSUM") as ps_proj, \
             tc.tile_pool(name="ps_sc", bufs=3, space="PSUM") as ps_sc, \
             tc.tile_pool(name="ps_ctx", bufs=2, space="PSUM") as ps_ctx:

            # ---- constant-ish inputs ----
            twkv = consts.tile([128, NIC, 128], F32R, tag="twkv", name="twkv")
            twq = consts.tile([128, NIC, DO], F32R, tag="twq", name="twq")
            for ic in range(NIC):
                nc.sync.dma_start(out=twkv[:, ic, :], in_=wkv[ic * 128:(ic + 1) * 128, :])
                nc.sync.dma_start(out=twq[:, ic, :], in_=wq[ic * 128:(ic + 1) * 128, :])
            tmd = consts.tile([128, 128], F32R, tag="tmd", name="tmd")
            tmp = consts.tile([128, 128], F32R, tag="tmp", name="tmp")
            tid = consts.tile([DO, DO], F32R, tag="tid", name="tid")
            nc.sync.dma_start(out=tmd, in_=mdiag[:, :])
            nc.sync.dma_start(out=tmp, in_=mpcol[:, :])
            nc.sync.dma_start(out=tid, in_=ident[:, :])

            # ---- x^T: 6 chunks x 2 column-halves, own key columns first ----
            xts = [[xpool.tile([128, S // 2], F32R, tag=f"xt_{ic}_{h}", name=f"xt_{ic}_{h}")
                    for h in range(2)] for ic in range(NIC)]
            for h in range(2):
                for ic in range(NIC):
                    nc.sync.dma_start(
                        out=xts[ic][h],
                        in_=xT[ic * 128:(ic + 1) * 128,
                               h * (S // 2):(h + 1) * (S // 2)])

            # ---- pass 1: [K^T | V^T] over own key columns (permuted [0, 2048)) ----
            kts = [qkv.tile([DO, QT], F32R, tag=f"kt_{st}", name=f"kt_{st}") for st in range(4)]
            vts = [qkv.tile([DO, QT], F32R, tag=f"vt_{st}", name=f"vt_{st}") for st in range(4)]
            for st in range(4):
                p1 = ps_proj.tile([128, QT], F32, tag="psproj", name="psproj")
                for ic in range(NIC):
                    nc.tensor.matmul(p1, twkv[:, ic, :],
                                     xts[ic][0][:, st * QT:(st + 1) * QT],
                                     start=(ic == 0), stop=(ic == NIC - 1))
                nc.vector.tensor_copy(kts[st], p1[0:DO, :])
                nc.vector.tensor_copy(vts[st], p1[DO:128, :])

            # ---- V^T -> V1 = [V | ones] per own key tile ----
            v1s = []
            for j in range(NOWN):
                st, col = j // 4, (j % 4) * 128
                pv = ps_proj.tile([128, DO], F32R, tag="psproj", name="psproj")
                nc.tensor.transpose(pv, vts[st][:, col:col + 128], tid)
                v1 = qkv.tile([128, DO + 1], F32R, tag=f"v1_{j}", name=f"v1_{j}")
                nc.vector.tensor_copy(v1[:, 0:DO], pv)
                # ones column for the row-sum (denominator); tmd[:,127] == 1
                nc.vector.tensor_copy(v1[:, DO:DO + 1], tmd[:, 127:128])
                v1s.append(v1)

            # ---- pass 2: Q^T over all (permuted) query columns ----
            qts = [qkv.tile([DO, QT], F32R, tag=f"qt_{st}", name=f"qt_{st}") for st in range(NQT)]
            for st in range(NQT):
                p2 = ps_proj.tile([128, QT], F32, tag="psproj", name="psproj")
                h, off = st // 4, (st % 4) * QT
                for ic in range(NIC):
                    nc.tensor.matmul(p2[0:DO, :], twq[:, ic, :],
                                     xts[ic][h][:, off:off + QT],
                                     start=(ic == 0), stop=(ic == NIC - 1))
                nc.vector.tensor_copy(qts[st], p2[0:DO, :])

            # ---- attention: per query tile T, accumulate num/den over key tiles ----
            for T in (0, 4, 1, 5, 2, 6, 3, 7):
                j0 = j0_of(T)
                nk = j0 + 4
                ctxp = ps_ctx.tile([DO + 1, QT], F32, tag="ctxp", name="ctxp")
                for j in range(nk):
                    r = j - j0
                    qlo = 128 * r if r > 0 else 0
                    w = QT - qlo
                    st, col = j // 4, (j % 4) * 128
                    sp = ps_sc.tile([128, QT], F32, tag="sp", name="sp")
                    nc.tensor.matmul(sp[:, 0:w], kts[st][:, col:col + 128],
                                     qts[T][:, qlo:QT], start=True, stop=True)
                    et = expp.tile([128, QT], F32R, tag="et", name="et")
                    nc.scalar.activation(et[:, 0:w], sp[:, 0:w],
                                         mybir.ActivationFunctionType.Exp,
                                         scale=float(1.0 / np.sqrt(DO)))
                    if r >= 0:
                        # boundary 128-col block: element-diagonal mask in the
                        # own-parity region, parity column mask in the other
                        nc.vector.tensor_mul(et[:, 0:128], et[:, 0:128],
                                             tmd if T < 4 else tmp)
                    nc.tensor.matmul(ctxp[:, qlo:QT], v1s[j], et[:, 0:w],
                                     start=(j == 0), stop=(j == nk - 1))
                ost = ndst.tile([DO + 1, QT], F32, tag="ost", name="ost")
                nc.vector.tensor_copy(ost, ctxp)
                nc.sync.dma_start(out=nd[:, T * QT:(T + 1) * QT], in_=ost)

    nc.compile()
    return nc


def get_program():
    if "nc" not in _prog_cache:
        _prog_cache["nc"] = build_program()
    return _prog_cache["nc"]


def core_perm(parity):
    """Permuted-to-global column index map: own key tiles first, then other."""
    own = [g for g in range(NKT) if g % 2 == parity]
    other = [g for g in range(NKT) if g % 2 != parity]
    return np.concatenate([np.arange(g * 128, (g + 1) * 128) for g in own + other])


def make_in_maps(x, Wq, Wk, Wv):
    x = np.asarray(x, dtype=np.float32)
    Wq = np.asarray(Wq, dtype=np.float32)
    Wk = np.asarray(Wk, dtype=np.float32)
    Wv = np.asarray(Wv, dtype=np.float32)
    wkv = np.concatenate([Wk, Wv], axis=1).copy()
    mdiag = np.triu(np.ones((128, 128), dtype=np.float32))  # keep k<=q: p<=f
    ident = np.eye(DO, dtype=np.float32)
    in_maps = []
    perms = []
    for c in range(NCORES):
        b, par = c // 2, c % 2
        perm = core_perm(par)
        perms.append(perm)
        xTp = np.ascontiguousarray(x[b].T[:, perm])
        mpcol = np.full((128, 128), 1.0 - par, dtype=np.float32)
        in_maps.append({
            "xT": xTp, "wkv": wkv, "wq": Wq,
            "mdiag": mdiag, "mpcol": mpcol, "ident": ident,
        })
    return in_maps, perms


def combine(results, perms):
    out = np.empty((B, S, DO), dtype=np.float32)
    for b in range(B):
        num = np.zeros((DO, S), dtype=np.float64)
        den = np.zeros((S,), dtype=np.float64)
        for c in (2 * b, 2 * b + 1):
            nd_c = results[c]["nd"].astype(np.float64)
            inv = np.empty(S, dtype=np.int64)
            inv[perms[c]] = np.arange(S)
            nd_g = nd_c[:, inv]
            num += nd_g[:DO]
            den += nd_g[DO]
        out[b] = (num / den).T.astype(np.float32)
    return out


def kernel(x, Wq, Wk, Wv):
    nc = get_program()
    in_maps, perms = make_in_maps(x, Wq, Wk, Wv)
    res = run_bass_kernel_spmd(nc, in_maps, list(range(NCORES)))
    return combine(res.results, perms)


# revision 7
# speedup vs baseline: 1.2681x; 1.2681x over previous
"""Causal attention (B=4, S=4096, D_IN=768, D_OUT=64) on 8 Trainium2 NeuronCores.

Sharding: core c handles batch b=c//2 and key-parity p=c%2 (the even or odd
128-wide key tiles of that batch). Every core computes, for ALL queries of its
batch, the unnormalized attention partials over its own key set:
    num[o, q] = sum_{k in own} exp(q.k/8) * V[k, o]
    den[q]    = sum_{k in own} exp(q.k/8)
The host sums the two partials per batch and normalizes: ctx = (num/den).T.
Causality is exact: key-tile work is skipped below the diagonal band and the
two boundary blocks are masked with host-provided mask tiles.

Host prep per core: x[b].T with columns permuted to [own key tiles | other key
tiles] so the device program is identical across cores (SPMD); masks and a
64x64 identity (for the on-chip V^T -> V transpose) are passed as inputs.

All matmuls run in float32r (single-pass fp32 on the PE at bf16 rate for
moving dims >= 256; ~1e-4 relative accuracy).
"""
import numpy as np

import concourse.bass as bass
import concourse.bacc as bacc
import concourse.tile as tile
from concourse import mybir
from concourse.bass_utils import run_bass_kernel_spmd

B, S, DI, DO = 4, 4096, 768, 64
NCORES = 8
NIC = DI // 128          # 6 contraction chunks
NKT = S // 128           # 32 global key tiles per batch
NOWN = NKT // 2          # 16 own key tiles per core
QT = 512                 # query tile width (one PSUM bank of fp32)
NQT = S // QT            # 8 query tiles
F32 = mybir.dt.float32
F32R = mybir.dt.float32r

_prog_cache = {}


def j0_of(T):
    """First diagonal-region packed key tile for permuted query tile T."""
    return 4 * T if T < 4 else 4 * (T - 4)


def build_program():
    """Build + compile the single SPMD Bass program (identical on all cores)."""
    nc = bacc.Bacc("TRN2", target_bir_lowering=False, debug=False)

    xT = nc.declare_dram_parameter("xT", [DI, S], F32R, isOutput=False)
    wkv = nc.declare_dram_parameter("wkv", [DI, 128], F32R, isOutput=False)
    # Wq zero-padded to [DI, 128] so Q^T comes out of PSUM with rows 64..127
    # already zero — the scores matmul then contracts over K=128 (the f32r
    # K=64 x M=128 shape runs at 2 cyc/row on HW; K=128 runs at ~1.06).
    wqp = nc.declare_dram_parameter("wqp", [DI, 128], F32R, isOutput=False)
    mdiag = nc.declare_dram_parameter("mdiag", [128, 128], F32R, isOutput=False)
    mpcol = nc.declare_dram_parameter("mpcol", [128, 128], F32R, isOutput=False)
    ident = nc.declare_dram_parameter("ident", [DO, DO], F32R, isOutput=False)
    nd = nc.declare_dram_parameter("nd", [DO + 1, S], F32, isOutput=True)

    with tile.TileContext(nc) as tc:
        with tc.tile_pool(name="consts", bufs=1) as consts, \
             tc.tile_pool(name="xpool", bufs=1) as xpool, \
             tc.tile_pool(name="qkv", bufs=1) as qkv, \
             tc.tile_pool(name="expp", bufs=6) as expp, \
             tc.tile_pool(name="ndst", bufs=3) as ndst, \
             tc.tile_pool(name="ps_proj", bufs=2, space="PSUM") as ps_proj, \
             tc.tile_pool(name="ps_sc2", bufs=2, space="PSUM") as ps_sc2, \
             tc.tile_pool(name="ps_ctx", bufs=2, space="PSUM") as ps_ctx:

            # ---- constant-ish inputs ----
            twkv = consts.tile([128, NIC, 128], F32R, tag="twkv", name="twkv")
            twq = consts.tile([128, NIC, 128], F32R, tag="twq", name="twq")
            for ic in range(NIC):
                nc.sync.dma_start(out=twkv[:, ic, :], in_=wkv[ic * 128:(ic + 1) * 128, :])
                nc.sync.dma_start(out=twq[:, ic, :], in_=wqp[ic * 128:(ic + 1) * 128, :])
            tmd = consts.tile([128, 128], F32R, tag="tmd", name="tmd")
            tmp = consts.tile([128, 128], F32R, tag="tmp", name="tmp")
            tid = consts.tile([DO, DO], F32R, tag="tid", name="tid")
            nc.sync.dma_start(out=tmd, in_=mdiag[:, :])
            nc.sync.dma_start(out=tmp, in_=mpcol[:, :])
            nc.sync.dma_start(out=tid, in_=ident[:, :])
            zsrc = consts.tile([DO, QT], F32, tag="zsrc", name="zsrc")
            nc.vector.memset(zsrc, 0.0)

            # ---- x^T in [128, 512] column tiles, column-major DMA order so the
            # first projection tile is ready after ~6 small DMAs, not the full load
            xc = [[xpool.tile([128, QT], F32R, tag=f"xc_{ic}_{cb}", name=f"xc_{ic}_{cb}")
                   for cb in range(NQT)] for ic in range(NIC)]
            for cb in range(NQT):
                for ic in range(NIC):
                    nc.sync.dma_start(
                        out=xc[ic][cb],
                        in_=xT[ic * 128:(ic + 1) * 128, cb * QT:(cb + 1) * QT])

            # ---- pass 1: [K^T | V^T] over own key columns (permuted [0, 2048)) ----
            # kt zero-padded to K=128 rows for the fast scores matmul shape
            kts = [qkv.tile([128, QT], F32R, tag=f"kt_{st}", name=f"kt_{st}") for st in range(4)]
            vts = [qkv.tile([DO, QT], F32R, tag=f"vt_{st}", name=f"vt_{st}") for st in range(4)]
            for st in range(4):
                p1 = ps_proj.tile([128, QT], F32, tag="psproj", name="psproj")
                for ic in range(NIC):
                    nc.tensor.matmul(p1, twkv[:, ic, :],
                                     xc[ic][st][:, :],
                                     start=(ic == 0), stop=(ic == NIC - 1))
                nc.vector.tensor_copy(kts[st][0:DO, :], p1[0:DO, :])
                nc.vector.tensor_copy(kts[st][DO:128, :], zsrc)
                nc.vector.tensor_copy(vts[st], p1[DO:128, :])

            # ---- V^T -> V1 = [V | ones] per own key tile ----
            v1s = []
            for j in range(NOWN):
                st, col = j // 4, (j % 4) * 128
                pv = ps_proj.tile([128, DO], F32R, tag="psproj", name="psproj")
                nc.tensor.transpose(pv, vts[st][:, col:col + 128], tid)
                v1 = qkv.tile([128, DO + 1], F32R, tag=f"v1_{j}", name=f"v1_{j}")
                nc.vector.tensor_copy(v1[:, 0:DO], pv)
                # ones column for the row-sum (denominator); tmd[:,127] == 1
                nc.vector.tensor_copy(v1[:, DO:DO + 1], tmd[:, 127:128])
                v1s.append(v1)

            # ---- pass 2: Q^T over all (permuted) query columns (rows 64.. zero) ----
            qts = [qkv.tile([128, QT], F32R, tag=f"qt_{st}", name=f"qt_{st}") for st in range(NQT)]
            for st in range(NQT):
                p2 = ps_proj.tile([128, QT], F32, tag="psproj", name="psproj")
                for ic in range(NIC):
                    nc.tensor.matmul(p2, twq[:, ic, :],
                                     xc[ic][st][:, :],
                                     start=(ic == 0), stop=(ic == NIC - 1))
                nc.vector.tensor_copy(qts[st], p2)

            # ---- attention: per query tile T, accumulate num/den over key tiles.
            # Full-width key tiles are processed in pairs sharing one 2-bank PSUM
            # tile and a single exp; the 4 diagonal-band tiles are packed 2+2.
            exp_scale = float(1.0 / np.sqrt(DO))

            def emit_scores(T, j, sp_ap, et_ap):
                """scores matmul for (T, j) into sp_ap ([128, w]), exp into et_ap."""
                r = j - j0_of(T)
                qlo = 128 * r if r > 0 else 0
                w = QT - qlo
                st, col = j // 4, (j % 4) * 128
                nc.tensor.matmul(sp_ap[:, 0:w], kts[st][:, col:col + 128],
                                 qts[T][:, qlo:QT], start=True, stop=True)
                return qlo, w

            for T in range(NQT):
                j0 = j0_of(T)
                nk = j0 + 4
                ctxp = ps_ctx.tile([DO + 1, QT], F32, tag="ctxp", name="ctxp")

                def ctx_mm(j, et_ap, qlo, w):
                    nc.tensor.matmul(ctxp[:, qlo:QT], v1s[j], et_ap[:, 0:w],
                                     start=(j == 0), stop=(j == nk - 1))

                # full-width tiles, paired
                for j in range(0, j0, 2):
                    sp2 = ps_sc2.tile([128, 2 * QT], F32, tag="sp2", name="sp2")
                    et2 = expp.tile([128, 2 * QT], F32R, tag="et", name="et")
                    emit_scores(T, j, sp2[:, 0:QT], None)
                    emit_scores(T, j + 1, sp2[:, QT:2 * QT], None)
                    nc.scalar.activation(et2, sp2,
                                         mybir.ActivationFunctionType.Exp,
                                         scale=exp_scale)
                    ctx_mm(j, et2[:, 0:QT], 0, QT)
                    ctx_mm(j + 1, et2[:, QT:2 * QT], 0, QT)
                # diagonal band: r=0 (w=512) + r=1 (w=384) share a 2-bank tile;
                # r=2 (w=256) + r=3 (w=128) share a 1-bank tile
                mask = tmd if T < 4 else tmp
                spb1 = ps_sc2.tile([128, 2 * QT], F32, tag="sp2", name="sp2")
                etb1 = expp.tile([128, 2 * QT], F32R, tag="et", name="et")
                emit_scores(T, j0, spb1[:, 0:QT], None)
                emit_scores(T, j0 + 1, spb1[:, QT:QT + 384], None)
                nc.scalar.activation(etb1[:, 0:QT + 384], spb1[:, 0:QT + 384],
                                     mybir.ActivationFunctionType.Exp,
                                     scale=exp_scale)
                nc.vector.tensor_mul(etb1[:, 0:128], etb1[:, 0:128], mask)
                nc.vector.tensor_mul(etb1[:, QT:QT + 128], etb1[:, QT:QT + 128], mask)
                ctx_mm(j0, etb1[:, 0:QT], 0, QT)
                ctx_mm(j0 + 1, etb1[:, QT:QT + 384], 128, 384)
                spb2 = ps_proj.tile([128, QT], F32, tag="psproj", name="psproj")
                etb2 = expp.tile([128, 2 * QT], F32R, tag="et", name="et")
                emit_scores(T, j0 + 2, spb2[:, 0:256], None)
                emit_scores(T, j0 + 3, spb2[:, 256:384], None)
                nc.scalar.activation(etb2[:, 0:384], spb2[:, 0:384],
                                     mybir.ActivationFunctionType.Exp,
                                     scale=exp_scale)
                nc.vector.tensor_mul(etb2[:, 0:128], etb2[:, 0:128], mask)
                nc.vector.tensor_mul(etb2[:, 256:384], etb2[:, 256:384], mask)
                ctx_mm(j0 + 2, etb2[:, 0:256], 256, 256)
                ctx_mm(j0 + 3, etb2[:, 256:384], 384, 128)

                ost = ndst.tile([DO + 1, QT], F32, tag="ost", name="ost")
                nc.vector.tensor_copy(ost, ctxp)
                nc.sync.dma_start(out=nd[:, T * QT:(T + 1) * QT], in_=ost)

    nc.compile()
    return nc


def get_program():
    if "nc" not in _prog_cache:
        _prog_cache["nc"] = build_program()
    return _prog_cache["nc"]


def core_perm(parity):
    """Permuted-to-global column index map: own key tiles first, then other."""
    own = [g for g in range(NKT) if g % 2 == parity]
    other = [g for g in range(NKT) if g % 2 != parity]
    return np.concatenate([np.arange(g * 128, (g + 1) * 128) for g in own + other])


def make_in_maps(x, Wq, Wk, Wv):
    x = np.asarray(x, dtype=np.float32)
    Wq = np.asarray(Wq, dtype=np.float32)
    Wk = np.asarray(Wk, dtype=np.float32)
    Wv = np.asarray(Wv, dtype=np.float32)
    wkv = np.concatenate([Wk, Wv], axis=1).copy()
    wqp = np.concatenate([Wq, np.zeros((DI, 128 - DO), np.float32)], axis=1).copy()
    mdiag = np.triu(np.ones((128, 128), dtype=np.float32))  # keep k<=q: p<=f
    ident = np.eye(DO, dtype=np.float32)
    in_maps = []
    perms = []
    for c in range(NCORES):
        b, par = c // 2, c % 2
        perm = core_perm(par)
        perms.append(perm)
        xTp = np.ascontiguousarray(x[b].T[:, perm])
        mpcol = np.full((128, 128), 1.0 - par, dtype=np.float32)
        in_maps.append({
            "xT": xTp, "wkv": wkv, "wqp": wqp,
            "mdiag": mdiag, "mpcol": mpcol, "ident": ident,
        })
    return in_maps, perms


def combine(results, perms):
    out = np.empty((B, S, DO), dtype=np.float32)
    for b in range(B):
        num = np.zeros((DO, S), dtype=np.float64)
        den = np.zeros((S,), dtype=np.float64)
        for c in (2 * b, 2 * b + 1):
            nd_c = results[c]["nd"].astype(np.float64)
            inv = np.empty(S, dtype=np.int64)
            inv[perms[c]] = np.arange(S)
            nd_g = nd_c[:, inv]
            num += nd_g[:DO]
            den += nd_g[DO]
        out[b] = (num / den).T.astype(np.float32)
    return out


def kernel(x, Wq, Wk, Wv):
    nc = get_program()
    in_maps, perms = make_in_maps(x, Wq, Wk, Wv)
    res = run_bass_kernel_spmd(nc, in_maps, list(range(NCORES)))
    return combine(res.results, perms)


# revision 11
# speedup vs baseline: 1.5704x; 1.2385x over previous
"""Causal attention (B=4, S=4096, D_IN=768, D_OUT=64) on 8 Trainium2 NeuronCores.

Sharding: core c handles batch b=c//2 and key-parity p=c%2 (the even or odd
128-wide key tiles of that batch). Every core computes, for ALL queries of its
batch, the unnormalized attention partials over its own key set:
    num[o, q] = sum_{k in own} exp(q.k/8) * V[k, o]
    den[q]    = sum_{k in own} exp(q.k/8)
The host sums the two partials per batch and normalizes: ctx = (num/den).T.
Causality is exact: key-tile work is skipped below the diagonal band and the
two boundary blocks are masked with host-provided mask tiles.

Host prep per core: x[b].T with columns permuted to [own key tiles | other key
tiles] so the device program is identical across cores (SPMD); masks and a
64x64 identity (for the on-chip V^T -> V transpose) are passed as inputs.

All matmuls run in float32r (single-pass fp32 on the PE at bf16 rate for
moving dims >= 256; ~1e-4 relative accuracy).
"""
import numpy as np

import concourse.bass as bass
import concourse.bacc as bacc
import concourse.tile as tile
from concourse import mybir
from concourse.bass_utils import run_bass_kernel_spmd

B, S, DI, DO = 4, 4096, 768, 64
NCORES = 8
NIC = DI // 128          # 6 contraction chunks
NKT = S // 128           # 32 global key tiles per batch
NOWN = NKT // 2          # 16 own key tiles per core
QT = 512                 # query tile width (one PSUM bank of fp32)
NQT = S // QT            # 8 query tiles
F32 = mybir.dt.float32
F32R = mybir.dt.float32r

_prog_cache = {}


def j0_of(T):
    """First diagonal-region packed key tile for permuted query tile T."""
    return 4 * T if T < 4 else 4 * (T - 4)


def build_program():
    """Build + compile the single SPMD Bass program (identical on all cores)."""
    nc = bacc.Bacc("TRN2", target_bir_lowering=False, debug=False)

    xT = nc.declare_dram_parameter("xT", [DI, S], F32R, isOutput=False)
    wkv = nc.declare_dram_parameter("wkv", [DI, 128], F32R, isOutput=False)
    # Wq zero-padded to [DI, 128] so Q^T comes out of PSUM with rows 64..127
    # already zero — the scores matmul then contracts over K=128 (the f32r
    # K=64 x M=128 shape runs at 2 cyc/row on HW; K=128 runs at ~1.06).
    wqp = nc.declare_dram_parameter("wqp", [DI, 128], F32R, isOutput=False)
    mdiag = nc.declare_dram_parameter("mdiag", [128, 128], F32R, isOutput=False)
    mpcol = nc.declare_dram_parameter("mpcol", [128, 128], F32R, isOutput=False)
    ident = nc.declare_dram_parameter("ident", [DO, DO], F32R, isOutput=False)
    nd = nc.declare_dram_parameter("nd", [DO + 1, S], F32, isOutput=True)

    with tile.TileContext(nc) as tc:
        with tc.tile_pool(name="consts", bufs=1) as consts, \
             tc.tile_pool(name="xpool", bufs=1) as xpool, \
             tc.tile_pool(name="qkv", bufs=1) as qkv, \
             tc.tile_pool(name="expp", bufs=6) as expp, \
             tc.tile_pool(name="ndst", bufs=3) as ndst, \
             tc.tile_pool(name="ps_proj", bufs=2, space="PSUM") as ps_proj, \
             tc.tile_pool(name="ps_sc2", bufs=2, space="PSUM") as ps_sc2, \
             tc.tile_pool(name="ps_ctx", bufs=2, space="PSUM") as ps_ctx:

            # ---- constant-ish inputs ----
            twkv = consts.tile([128, NIC, 128], F32R, tag="twkv", name="twkv")
            twq = consts.tile([128, NIC, 128], F32R, tag="twq", name="twq")
            nc.sync.dma_start(out=twkv, in_=wkv.rearrange("(c p) w -> p c w", p=128))
            nc.sync.dma_start(out=twq, in_=wqp.rearrange("(c p) w -> p c w", p=128))
            tmd = consts.tile([128, 128], F32R, tag="tmd", name="tmd")
            tmp = consts.tile([128, 128], F32R, tag="tmp", name="tmp")
            tid = consts.tile([DO, DO], F32R, tag="tid", name="tid")
            nc.sync.dma_start(out=tmd, in_=mdiag[:, :])
            nc.sync.dma_start(out=tmp, in_=mpcol[:, :])
            nc.sync.dma_start(out=tid, in_=ident[:, :])
            zsrc = consts.tile([DO, QT], F32, tag="zsrc", name="zsrc")
            nc.vector.memset(zsrc, 0.0)

            # ---- x^T in [128, 1024] column-pair tiles, column-major DMA order so
            # the first projection tiles are ready after ~3MB, not the full load
            xcp = [[xpool.tile([128, 2 * QT], F32R, tag=f"xcp_{ic}_{cq}", name=f"xcp_{ic}_{cq}")
                    for cq in range(NQT // 2)] for ic in range(NIC)]
            for cq in range(NQT // 2):
                for ic in range(NIC):
                    nc.sync.dma_start(
                        out=xcp[ic][cq],
                        in_=xT[ic * 128:(ic + 1) * 128,
                               cq * 2 * QT:(cq + 1) * 2 * QT])

            def xc(ic, cb):
                """[128, 512] view of column block cb inside its column-pair tile."""
                return xcp[ic][cb // 2][:, (cb % 2) * QT:(cb % 2 + 1) * QT]

            # ---- pass 1: [K^T | V^T] over own key columns (permuted [0, 2048)) ----
            # kt zero-padded to K=128 rows for the fast scores matmul shape
            kts = [qkv.tile([128, QT], F32R, tag=f"kt_{st}", name=f"kt_{st}") for st in range(4)]
            vts = [qkv.tile([DO, QT], F32R, tag=f"vt_{st}", name=f"vt_{st}") for st in range(4)]
            for st in range(4):
                p1 = ps_proj.tile([128, QT], F32, tag="psproj", name="psproj")
                for ic in range(NIC):
                    nc.tensor.matmul(p1, twkv[:, ic, :],
                                     xc(ic, st),
                                     start=(ic == 0), stop=(ic == NIC - 1))
                nc.vector.tensor_copy(kts[st][0:DO, :], p1[0:DO, :])
                nc.vector.tensor_copy(kts[st][DO:128, :], zsrc)
                nc.vector.tensor_copy(vts[st], p1[DO:128, :])

            # ---- V^T -> V1 = [V | ones] per own key tile ----
            v1s = []
            for j in range(NOWN):
                st, col = j // 4, (j % 4) * 128
                pv = ps_proj.tile([128, DO], F32R, tag="psproj", name="psproj")
                nc.tensor.transpose(pv, vts[st][:, col:col + 128], tid)
                v1 = qkv.tile([128, DO + 1], F32R, tag=f"v1_{j}", name=f"v1_{j}")
                nc.vector.tensor_copy(v1[:, 0:DO], pv)
                # ones column for the row-sum (denominator); tmd[:,127] == 1
                nc.vector.tensor_copy(v1[:, DO:DO + 1], tmd[:, 127:128])
                v1s.append(v1)

            # ---- pass 2: Q^T over all (permuted) query columns (rows 64.. zero) ----
            qts = [qkv.tile([128, QT], F32R, tag=f"qt_{st}", name=f"qt_{st}") for st in range(NQT)]
            for st in range(NQT):
                p2 = ps_proj.tile([128, QT], F32, tag="psproj", name="psproj")
                for ic in range(NIC):
                    nc.tensor.matmul(p2, twq[:, ic, :],
                                     xc(ic, st),
                                     start=(ic == 0), stop=(ic == NIC - 1))
                nc.vector.tensor_copy(qts[st], p2)

            # ---- attention: per query tile T, accumulate num/den over key tiles.
            # Full-width key tiles are processed in pairs sharing one 2-bank PSUM
            # tile and a single exp; the 4 diagonal-band tiles are packed 2+2.
            exp_scale = float(1.0 / np.sqrt(DO))

            def emit_scores(T, j, sp_ap, et_ap):
                """scores matmul for (T, j) into sp_ap ([128, w]), exp into et_ap."""
                r = j - j0_of(T)
                qlo = 128 * r if r > 0 else 0
                w = QT - qlo
                st, col = j // 4, (j % 4) * 128
                nc.tensor.matmul(sp_ap[:, 0:w], kts[st][:, col:col + 128],
                                 qts[T][:, qlo:QT], start=True, stop=True)
                return qlo, w

            for T in range(NQT):
                j0 = j0_of(T)
                nk = j0 + 4
                mask = tmd if T < 4 else tmp
                # Phase A: all scores matmuls + exps (+ boundary masks) for this
                # query tile. Emitting these before any ctx matmul keeps the PE
                # queue free of loads that wait on the scalar engine's exps.
                ctx_args = []   # (j, et_ap, qlo, w) consumed in phase B
                for j in range(0, j0, 2):
                    sp2 = ps_sc2.tile([128, 2 * QT], F32, tag="sp2", name="sp2")
                    et2 = expp.tile([128, 2 * QT], F32R, tag="et", name="et")
                    emit_scores(T, j, sp2[:, 0:QT], None)
                    emit_scores(T, j + 1, sp2[:, QT:2 * QT], None)
                    nc.scalar.activation(et2, sp2,
                                         mybir.ActivationFunctionType.Exp,
                                         scale=exp_scale)
                    ctx_args.append((j, et2[:, 0:QT], 0, QT))
                    ctx_args.append((j + 1, et2[:, QT:2 * QT], 0, QT))
                # diagonal band: r=0 (w=512) + r=1 (w=384) share a 2-bank tile;
                # r=2 (w=256) + r=3 (w=128) share a 1-bank tile
                spb1 = ps_sc2.tile([128, 2 * QT], F32, tag="sp2", name="sp2")
                etb1 = expp.tile([128, 2 * QT], F32R, tag="et", name="et")
                emit_scores(T, j0, spb1[:, 0:QT], None)
                emit_scores(T, j0 + 1, spb1[:, QT:QT + 384], None)
                nc.scalar.activation(etb1[:, 0:QT + 384], spb1[:, 0:QT + 384],
                                     mybir.ActivationFunctionType.Exp,
                                     scale=exp_scale)
                nc.vector.tensor_mul(etb1[:, 0:128], etb1[:, 0:128], mask)
                nc.vector.tensor_mul(etb1[:, QT:QT + 128], etb1[:, QT:QT + 128], mask)
                ctx_args.append((j0, etb1[:, 0:QT], 0, QT))
                ctx_args.append((j0 + 1, etb1[:, QT:QT + 384], 128, 384))
                spb2 = ps_proj.tile([128, QT], F32, tag="psproj", name="psproj")
                etb2 = expp.tile([128, 2 * QT], F32R, tag="et", name="et")
                emit_scores(T, j0 + 2, spb2[:, 0:256], None)
                emit_scores(T, j0 + 3, spb2[:, 256:384], None)
                nc.scalar.activation(etb2[:, 0:384], spb2[:, 0:384],
                                     mybir.ActivationFunctionType.Exp,
                                     scale=exp_scale)
                nc.vector.tensor_mul(etb2[:, 0:128], etb2[:, 0:128], mask)
                nc.vector.tensor_mul(etb2[:, 256:384], etb2[:, 256:384], mask)
                ctx_args.append((j0 + 2, etb2[:, 0:256], 256, 256))
                ctx_args.append((j0 + 3, etb2[:, 256:384], 384, 128))

                # Phase B: accumulate num/den
                ctxp = ps_ctx.tile([DO + 1, QT], F32, tag="ctxp", name="ctxp")
                for j, et_ap, qlo, w in ctx_args:
                    nc.tensor.matmul(ctxp[:, qlo:QT], v1s[j], et_ap[:, 0:w],
                                     start=(j == 0), stop=(j == nk - 1))

                ost = ndst.tile([DO + 1, QT], F32, tag="ost", name="ost")
                nc.vector.tensor_copy(ost, ctxp)
                nc.sync.dma_start(out=nd[:, T * QT:(T + 1) * QT], in_=ost)

    nc.compile()
    return nc


def get_program():
    if "nc" not in _prog_cache:
        _prog_cache["nc"] = build_program()
    return _prog_cache["nc"]


def core_perm(parity):
    """Permuted-to-global column index map: own key tiles first, then other."""
    own = [g for g in range(NKT) if g % 2 == parity]
    other = [g for g in range(NKT) if g % 2 != parity]
    return np.concatenate([np.arange(g * 128, (g + 1) * 128) for g in own + other])


def make_in_maps(x, Wq, Wk, Wv):
    x = np.asarray(x, dtype=np.float32)
    Wq = np.asarray(Wq, dtype=np.float32)
    Wk = np.asarray(Wk, dtype=np.float32)
    Wv = np.asarray(Wv, dtype=np.float32)
    wkv = np.concatenate([Wk, Wv], axis=1).copy()
    wqp = np.concatenate([Wq, np.zeros((DI, 128 - DO), np.float32)], axis=1).copy()
    mdiag = np.triu(np.ones((128, 128), dtype=np.float32))  # keep k<=q: p<=f
    ident = np.eye(DO, dtype=np.float32)
    in_maps = []
    perms = []
    for c in range(NCORES):
        b, par = c // 2, c % 2
        perm = core_perm(par)
        perms.append(perm)
        xTp = np.ascontiguousarray(x[b].T[:, perm])
        mpcol = np.full((128, 128), 1.0 - par, dtype=np.float32)
        in_maps.append({
            "xT": xTp, "wkv": wkv, "wqp": wqp,
            "mdiag": mdiag, "mpcol": mpcol, "ident": ident,
        })
    return in_maps, perms


def combine(results, perms):
    out = np.empty((B, S, DO), dtype=np.float32)
    for b in range(B):
        num = np.zeros((DO, S), dtype=np.float64)
        den = np.zeros((S,), dtype=np.float64)
        for c in (2 * b, 2 * b + 1):
            nd_c = results[c]["nd"].astype(np.float64)
            inv = np.empty(S, dtype=np.int64)
            inv[perms[c]] = np.arange(S)
            nd_g = nd_c[:, inv]
            num += nd_g[:DO]
            den += nd_g[DO]
        out[b] = (num / den).T.astype(np.float32)
    return out


def kernel(x, Wq, Wk, Wv):
    nc = get_program()
    in_maps, perms = make_in_maps(x, Wq, Wk, Wv)
    res = run_bass_kernel_spmd(nc, in_maps, list(range(NCORES)))
    return combine(res.results, perms)
